# revision 23
# baseline (speedup 1.0000x reference)
"""Causal multi-head attention on 8 Trainium2 NeuronCores.

Problem (hardcoded): x [4, 2048, 1024] fp32, W_qkv [1024, 3072], b_qkv [3072],
W_o [1024, 1024], b_o [1024]; 16 heads, head_dim 64.

Sharding: 8 cores = 4 batches x 2 head-groups (8 heads each). Each core
computes QKV projection for its (batch, head-group), causal attention for its
8 heads, and a partial out-projection [2048, 1024]. Host sums the two
head-group partials per batch and adds b_o.

Kernel strategy (per core, "transposed" domain):
  - x strip [512, 1024] -> PE-transpose -> xT [128, 8ds, 512]
  - QT/KTz = W^T x^T via matmul(lhsT=W_tile, rhs=xT); KTz zero-padded per
    head so the score matmul contracts K=128 (keeps PE at full clock).
  - V natural = matmul(lhsT=xT_tile, rhs=Wv), stored [128, blk, head, 65]
    with a ones column (denominator accumulates in psO row 64).
  - Scores per (head, strip): sk-blocks processed in PAIRS sharing one
    [128,1024] 2-bank psum tile; ONE Exp activation per off-diagonal pair.
    Diagonal blocks are causally trimmed (A/exp/AV restricted to sq>=128j)
    and masked via one strided 2-corner multiply with a [128,128] triangle.
  - Normalize: denominator copy + reciprocal_approx_fast + gpsimd
    partition_broadcast; OT = psO * recip (DVE).
  - out partial = matmul(lhsT=OT tile, rhs=W_o tiles) -> [s, e] -> DMA out.
  - Software pipelining: transposes/QKV-proj of strip i+1 and out-proj of
    strip i-1 are interleaved as PE fillers between attention pairs, so the
    PE fills gaps while Scalar (Exp) paces the attention inner loop.
Projection/out-proj matmuls run float32r; attention matmuls run bf16.
"""

import ml_dtypes
import numpy as np

import concourse.bass as bass
from concourse import bacc
import concourse.mybir as mybir
from concourse.bass_utils import run_bass_kernel_spmd
from concourse.tile import TileContext

B, S, D = 4, 2048, 1024
H, HD = 16, 64
G = 2                  # head groups (cores per batch)
HPG = H // G           # 8 heads per core
NG = HPG * HD          # 512 qkv feature columns per core
N_CORES = 8
STRIP = 512            # sq strip width
NSTRIP = S // STRIP    # 4
DS = D // 128          # 8 contraction subtiles for the projections
FP32 = mybir.dt.float32
R32 = mybir.dt.float32r
BF16 = mybir.dt.bfloat16
AF = mybir.ActivationFunctionType


def build_bass(dbg=False):
    nc = bacc.Bacc("TRN2")

    xt_d = nc.dram_tensor("xt", [D, S], R32, kind="ExternalInput")
    wq_d = nc.dram_tensor("wq", [128, 2, DS, NG // 2], R32, kind="ExternalInput")
    wk_d = nc.dram_tensor("wk", [128, 2, DS, NG // 2], R32, kind="ExternalInput")
    wv_d = nc.dram_tensor("wv", [128, DS, NG], R32, kind="ExternalInput")
    bqk_d = nc.dram_tensor("bqk", [128, 8], FP32, kind="ExternalInput")
    tri_d = nc.dram_tensor("tri", [128, 2, 128], BF16, kind="ExternalInput")
    bv_d = nc.dram_tensor("bv", [1, NG], FP32, kind="ExternalInput")
    wo_d = nc.dram_tensor("wo", [128, 4, D], R32, kind="ExternalInput")
    out_d = nc.dram_tensor("out", [S, D], FP32, kind="ExternalOutput")

    with TileContext(nc) as tc:
        with (
            tc.tile_pool(name="const", bufs=1) as const,
            tc.tile_pool(name="persist", bufs=1) as persist,
            tc.tile_pool(name="work", bufs=2) as work,
            tc.tile_pool(name="psum", bufs=2, space="PSUM") as psum,
        ):
            xT = {}      # strip -> tile

            def emit_xT_dmas(i):
                # x arrives pre-transposed from the host: per-ds DMA slices
                s0 = i * STRIP
                xT[i] = work.tile([128, DS, STRIP], R32, name="xT",
                                  tag="xT", bufs=2)
                for ds in range(DS):
                    nc.sync.dma_start(
                        xT[i][:, ds, :],
                        xt_d[ds * 128:(ds + 1) * 128, s0:s0 + STRIP])

            emit_xT_dmas(0)
            tri2 = const.tile([128, 2, 128], BF16, name="tri2")
            nc.sync.dma_start(tri2, tri_d[:, :, :])
            bqk_sb = const.tile([128, 8], FP32, name="bqk_sb")
            nc.sync.dma_start(bqk_sb, bqk_d[:, :])
            bv_sb = const.tile([1, NG], FP32, name="bv_sb")
            nc.sync.dma_start(bv_sb, bv_d[:, :])
            # bias broadcast for the Vn evacuation add (one-time, on gpsimd)
            bvb = const.tile([128, NG], FP32, name="bvb")
            nc.gpsimd.partition_broadcast(bvb, bv_sb[0:1, :])
            # weights pre-rearranged on host to [128, ...] (contiguous per
            # partition -> full-BW DMA, cheap descriptors). wq/wk stream as
            # interleaved halves on the Activation HWDGE queue so the first
            # K-projection unblocks early; wv rides the sync queue after
            # the strip-0 x tiles; wo (needed last) trails on the act queue
            wq_sb = const.tile([128, 2, DS, NG // 2], R32, name="wq_sb")
            wk_sb = const.tile([128, 2, DS, NG // 2], R32, name="wk_sb")
            for hf in range(2):
                nc.scalar.dma_start(wq_sb[:, hf], wq_d[:, hf])
                nc.scalar.dma_start(wk_sb[:, hf], wk_d[:, hf])
            wv_sb = const.tile([128, DS, NG], R32, name="wv_sb")
            nc.scalar.dma_start(wv_sb, wv_d[:, :, :])
            wo_sb = const.tile([128, 4, D], R32, name="wo_sb")
            nc.scalar.dma_start(wo_sb, wo_d[:, :, :])

            # Persistent zero-padded K^T per head and V tiles (both bf16)
            KTz = persist.tile([128, HPG, S], BF16, name="KTz")
            # even heads occupy rows 0-63 (zero 64-127); odd heads vice versa
            for h in range(HPG):
                zrow = 64 if h % 2 == 0 else 0
                nc.gpsimd.memset(KTz[zrow:zrow + 64, h, :], 0.0)
            Vn = persist.tile([128, S // 128, HPG, HD + 1], BF16, name="Vn")
            nc.gpsimd.memset(Vn[:, :, :, HD], 1.0)

            QT = {}      # strip -> tile
            OT = {}      # strip -> tile
            ob = {}      # strip -> tile

            def qk_chunk(i, which, nb):
                # 8 matmuls (full D contraction) + bias-add evacuation
                s0 = i * STRIP
                if which == 0 and nb == 0:
                    QT[i] = work.tile([128, 4, STRIP], BF16, name="QT",
                                      tag="QT", bufs=2)
                w_sb = wq_sb if which == 0 else wk_sb
                hf, c0 = nb // 2, (nb % 2) * 128
                ps = psum.tile([128, STRIP], FP32, name="ps", tag="ps_mm",
                               bufs=2)
                for ds in range(DS):
                    nc.tensor.matmul(
                        ps, lhsT=w_sb[:, hf, ds, c0:c0 + 128],
                        rhs=xT[i][:, ds],
                        start=(ds == 0), stop=(ds == DS - 1))
                bcol = bqk_sb[:, 4 * which + nb:4 * which + nb + 1]
                # Q/K evacuations on Scalar (Identity shares Exp's act table)
                # to keep the Vector queue shallow for the norm chains
                if which == 0:
                    nc.scalar.activation(QT[i][:, nb, :], ps, AF.Identity,
                                         bias=bcol)
                else:
                    nc.scalar.activation(
                        KTz[0:64, 2 * nb, s0:s0 + STRIP],
                        ps[0:64, :], AF.Identity, bias=bcol[0:64, :])
                    nc.scalar.activation(
                        KTz[64:128, 2 * nb + 1, s0:s0 + STRIP],
                        ps[64:128, :], AF.Identity, bias=bcol[64:128, :])

            def v_chunk(i, st):
                stg = i * 4 + st
                ps = psum.tile([128, STRIP], FP32, name="psv", tag="ps_mm",
                               bufs=2)
                for ds in range(DS):
                    nc.tensor.matmul(
                        ps,
                        lhsT=xT[i][:, ds, st * 128:(st + 1) * 128],
                        rhs=wv_sb[:, ds],
                        start=(ds == 0), stop=(ds == DS - 1))
                nc.vector.tensor_add(
                    Vn[:, stg, :, 0:HD],
                    ps.rearrange("p (h d) -> p h d", d=HD),
                    bvb.rearrange("p (h d) -> p h d", d=HD))

            ops_open = {}  # (i, st, ec) -> psum tile across half-chunks

            def outproj_chunk(i, st, ec, nh, evac="v"):
                # half-chunk: ns 0-1 (nh=0) opens the psum group, ns 2-3
                # (nh=1) closes it and evacuates on gpsimd
                if (st, ec, nh) == (0, 0, 0) and i not in ob:
                    ob[i] = work.tile([128, D], FP32, name="ob", tag="ob",
                                      bufs=2)
                s0 = i * STRIP
                if nh == 0:
                    ps = psum.tile([128, STRIP], FP32, name="pso",
                                   tag="ps_mm", bufs=2)
                    ops_open[(i, st, ec)] = ps
                else:
                    ps = ops_open.pop((i, st, ec))
                for ns in (2 * nh, 2 * nh + 1):
                    nc.tensor.matmul(
                        ps,
                        lhsT=OT[i][:, ns, st * 128:(st + 1) * 128],
                        rhs=wo_sb[:, ns, ec * 512:(ec + 1) * 512],
                        start=(ns == 0), stop=(ns == 3))
                if nh == 1:
                    if evac == "s":
                        nc.scalar.copy(ob[i][:, ec * 512:(ec + 1) * 512], ps)
                    else:
                        nc.vector.tensor_copy(
                            ob[i][:, ec * 512:(ec + 1) * 512], ps)
                    if ec == 1:
                        nc.scalar.dma_start(
                            out_d[s0 + st * 128:s0 + (st + 1) * 128, :],
                            ob[i])

            def attention(i, fillers):
                def fill():
                    try:
                        next(fillers)()
                    except StopIteration:
                        pass

                npair = 2 * i + 2  # 2i off-diagonal pairs + 2 diagonal pairs
                OT[i] = work.tile([128, 4, STRIP], R32, name="OT", tag="OT",
                                  bufs=3)
                for h in range(HPG):
                    prow = (h % 2) * 64
                    nsub = h // 2
                    psO = psum.tile([128, STRIP], FP32, name="psO", tag="psO",
                                    bufs=2)
                    for p in range(npair):
                        psA = psum.tile([128, 1024], FP32, name="psA",
                                        tag="psA2", bufs=2)
                        expP = work.tile([128, 1024], BF16, name="expP",
                                         tag="expP", bufs=4)
                        if p < 2 * i:        # off-diagonal pair, full width
                            for l in range(2):
                                b = 2 * p + l
                                nc.tensor.matmul(
                                    psA[:, 512 * l:512 * (l + 1)],
                                    lhsT=KTz[:, h, b * 128:(b + 1) * 128],
                                    rhs=QT[i][:, nsub, :],
                                    start=True, stop=True)
                            nc.scalar.activation(expP, psA, AF.Exp,
                                                 scale=0.125)
                            for l in range(2):
                                b = 2 * p + l
                                nc.tensor.matmul(
                                    psO[0:HD + 1, :],
                                    lhsT=Vn[:, b, h, :],
                                    rhs=expP[:, 512 * l:512 * (l + 1)],
                                    start=(p == 0 and l == 0), stop=False,
                                    skip_group_check=True)
                        else:                # diagonal pair, causally trimmed
                            pd = p - 2 * i
                            for l in range(2):
                                j = 2 * pd + l
                                b = 4 * i + j
                                c0 = 512 * l + 128 * j
                                nc.tensor.matmul(
                                    psA[:, c0:512 * (l + 1)],
                                    lhsT=KTz[:, h, b * 128:(b + 1) * 128],
                                    rhs=QT[i][:, nsub, 128 * j:STRIP],
                                    start=True, stop=True,
                                    skip_group_check=True)
                            # one exp spanning both trimmed slots; the stale
                            # psum columns in between are exp'd but never read
                            nc.scalar.activation(
                                expP[:, 256 * pd:1024],
                                psA[:, 256 * pd:1024],
                                AF.Exp, scale=0.125)
                            # strided 2-corner causal mask multiply
                            cbase = expP[:, 256 * pd:256 * pd + 768]
                            cap = bass.AP(
                                tensor=cbase.tensor, offset=cbase.offset,
                                ap=[list(cbase.ap[0])] + [[640, 2], [1, 128]])
                            nc.vector.tensor_mul(cap, cap, tri2)
                            for l in range(2):
                                j = 2 * pd + l
                                b = 4 * i + j
                                c0 = 512 * l + 128 * j
                                nc.tensor.matmul(
                                    psO[0:HD + 1, 128 * j:STRIP],
                                    lhsT=Vn[:, b, h, :],
                                    rhs=expP[:, c0:512 * (l + 1)],
                                    start=(i == 0 and p == 0 and l == 0),
                                    stop=(p == npair - 1 and l == 1),
                                    skip_group_check=True)
                            fill()   # diag pairs are scalar-paced: extra fill
                        fill()
                    # normalize: recip of denominator row, broadcast, mult
                    den = work.tile([1, STRIP], FP32, name="den", tag="den",
                                    bufs=1)
                    nc.vector.tensor_copy(den, psO[HD:HD + 1, :])
                    recip = work.tile([1, STRIP], FP32, name="recip",
                                      tag="recip", bufs=1)
                    nc.vector.reciprocal_approx_fast(recip, den)
                    pbt = work.tile([64, STRIP], FP32, name="pbt", tag="pbt",
                                    bufs=2)
                    nc.gpsimd.partition_broadcast(pbt, recip[0:1, :])
                    nc.vector.tensor_mul(OT[i][prow:prow + 64, nsub, :],
                                         psO[0:HD, :], pbt)
                    fill()

            # ---- prologue: strip 0 projections ----
            for which in range(2):
                for nb in range(4):
                    qk_chunk(0, which, nb)
            for st in range(4):
                v_chunk(0, st)

            # ---- main loop: attention(i) with interleaved fillers ----
            for i in range(NSTRIP):
                fillers = []
                if i + 1 < NSTRIP:
                    emit_xT_dmas(i + 1)
                    for which in range(2):
                        for nb in range(4):
                            fillers.append(
                                lambda which=which, nb=nb:
                                qk_chunk(i + 1, which, nb))
                    for st in range(4):
                        fillers.append(lambda st=st: v_chunk(i + 1, st))
                # out-proj chunks available this strip: first half of the
                # previous strip's, deferred half of the one before (keeps
                # attention(3), which has no proj fillers, supplied with PE
                # work). Halves of one psum group stay adjacent.
                opc = []
                if i >= 1:
                    sts = (0, 1) if i < NSTRIP - 1 else (0, 1, 2, 3)
                    opc += [(i - 1, st, ec) for st in sts for ec in range(2)]
                if i >= 2:
                    opc += [(i - 2, st, ec) for st in (2, 3) for ec in range(2)]
                if opc:
                    mixed = []
                    fi = iter(fillers)
                    for ii, st, ec in opc:
                        mixed.append(lambda ii=ii, st=st, ec=ec:
                                     outproj_chunk(ii, st, ec, 0))
                        mixed.append(lambda ii=ii, st=st, ec=ec:
                                     outproj_chunk(ii, st, ec, 1))
                        for _ in range(2):
                            try:
                                mixed.append(next(fi))
                            except StopIteration:
                                break
                    mixed.extend(fi)
                    fillers = mixed
                fit = iter(fillers)
                attention(i, fit)
                for f in fit:   # leftover fillers
                    f()

            # ---- final strip out-projection (evacs split across
            # Scalar+Vector: both are idle here and the ps_mm rotation is
            # latency-bound on the evacuation) ----
            for st in range(4):
                for ec in range(2):
                    outproj_chunk(NSTRIP - 1, st, ec, 0)
                    outproj_chunk(NSTRIP - 1, st, ec, 1,
                                  evac="s" if ec == 0 else "v")
    nc.compile()
    return nc


_CACHE = {}


def _tri_mask():
    # T[p, l, c] = 1.0 if c >= p else 0 (keep sq >= sk on diagonal corners)
    p = np.arange(128)[:, None, None]
    c = np.arange(128)[None, None, :]
    return np.broadcast_to(
        (c >= p), (128, 2, 128)).astype(np.float32).astype(ml_dtypes.bfloat16)


def kernel(x, W_qkv, b_qkv, W_o, b_o):
    x = np.ascontiguousarray(np.asarray(x, dtype=np.float32))
    W_qkv = np.asarray(W_qkv, dtype=np.float32)
    b_qkv = np.asarray(b_qkv, dtype=np.float32)
    W_o = np.asarray(W_o, dtype=np.float32)
    b_o = np.asarray(b_o, dtype=np.float32)

    if "nc" not in _CACHE:
        _CACHE["nc"] = build_bass()
    nc = _CACHE["nc"]

    in_maps = []
    for c in range(N_CORES):
        b, g = c // G, c % G
        n0 = g * NG
        bq = b_qkv[n0:n0 + NG]
        bk = b_qkv[D + n0:D + n0 + NG]
        bqk = np.concatenate(
            [bq.reshape(4, 128).T, bk.reshape(4, 128).T], axis=1)  # [128, 8]
        def _w(m):  # [D, NG] -> [128, DS, NG] contiguous
            return np.ascontiguousarray(
                m.reshape(DS, 128, -1).transpose(1, 0, 2))

        def _wh(m):  # [D, NG] -> [128, 2, DS, NG//2] (n-halves contiguous)
            r = m.reshape(DS, 128, 2, NG // 2)
            return np.ascontiguousarray(r.transpose(1, 2, 0, 3))
        in_maps.append({
            "xt": np.ascontiguousarray(x[b].T),
            "wq": _wh(W_qkv[:, n0:n0 + NG]),
            "wk": _wh(W_qkv[:, D + n0:D + n0 + NG]),
            "wv": _w(W_qkv[:, 2 * D + n0:2 * D + n0 + NG]),
            "bqk": np.ascontiguousarray(bqk),
            "bv": np.ascontiguousarray(
                b_qkv[2 * D + n0:2 * D + n0 + NG].reshape(1, NG)),
            "wo": np.ascontiguousarray(
                W_o[n0:n0 + NG, :].reshape(4, 128, D).transpose(1, 0, 2)),
            "tri": _tri_mask(),
        })

    _CACHE["in_maps"] = in_maps
    res = run_bass_kernel_spmd(nc, in_maps, list(range(N_CORES)))
    outs = res.results

    out = np.empty((B, S, D), dtype=np.float32)
    for b in range(B):
        out[b] = outs[G * b]["out"] + outs[G * b + 1]["out"]
    out += b_o[None, None, :]
    return out


# revision 24
# speedup vs baseline: 1.0020x; 1.0020x over previous
"""Causal multi-head attention on 8 Trainium2 NeuronCores.

Problem (hardcoded): x [4, 2048, 1024] fp32, W_qkv [1024, 3072], b_qkv [3072],
W_o [1024, 1024], b_o [1024]; 16 heads, head_dim 64.

Sharding: 8 cores = 4 batches x 2 head-groups (8 heads each). Each core
computes QKV projection for its (batch, head-group), causal attention for its
8 heads, and a partial out-projection [2048, 1024]. Host sums the two
head-group partials per batch and adds b_o.

Kernel strategy (per core, "transposed" domain):
  - x strip [512, 1024] -> PE-transpose -> xT [128, 8ds, 512]
  - QT/KTz = W^T x^T via matmul(lhsT=W_tile, rhs=xT); KTz zero-padded per
    head so the score matmul contracts K=128 (keeps PE at full clock).
  - V natural = matmul(lhsT=xT_tile, rhs=Wv), stored [128, blk, head, 65]
    with a ones column (denominator accumulates in psO row 64).
  - Scores per (head, strip): sk-blocks processed in PAIRS sharing one
    [128,1024] 2-bank psum tile; ONE Exp activation per off-diagonal pair.
    Diagonal blocks are causally trimmed (A/exp/AV restricted to sq>=128j)
    and masked via one strided 2-corner multiply with a [128,128] triangle.
  - Normalize: denominator copy + reciprocal_approx_fast + gpsimd
    partition_broadcast; OT = psO * recip (DVE).
  - out partial = matmul(lhsT=OT tile, rhs=W_o tiles) -> [s, e] -> DMA out.
  - Software pipelining: transposes/QKV-proj of strip i+1 and out-proj of
    strip i-1 are interleaved as PE fillers between attention pairs, so the
    PE fills gaps while Scalar (Exp) paces the attention inner loop.
Projection/out-proj matmuls run float32r; attention matmuls run bf16.
"""

import ml_dtypes
import numpy as np

import concourse.bass as bass
from concourse import bacc
import concourse.mybir as mybir
from concourse.bass_utils import run_bass_kernel_spmd
from concourse.tile import TileContext

B, S, D = 4, 2048, 1024
H, HD = 16, 64
G = 2                  # head groups (cores per batch)
HPG = H // G           # 8 heads per core
NG = HPG * HD          # 512 qkv feature columns per core
N_CORES = 8
STRIP = 512            # sq strip width
NSTRIP = S // STRIP    # 4
DS = D // 128          # 8 contraction subtiles for the projections
FP32 = mybir.dt.float32
R32 = mybir.dt.float32r
BF16 = mybir.dt.bfloat16
AF = mybir.ActivationFunctionType


def build_bass(dbg=False):
    nc = bacc.Bacc("TRN2")

    xt_d = nc.dram_tensor("xt", [D, S], R32, kind="ExternalInput")
    wq_d = nc.dram_tensor("wq", [128, 2, DS, NG // 2], R32, kind="ExternalInput")
    wk_d = nc.dram_tensor("wk", [128, 2, DS, NG // 2], R32, kind="ExternalInput")
    wv_d = nc.dram_tensor("wv", [128, DS, NG], R32, kind="ExternalInput")
    bqk_d = nc.dram_tensor("bqk", [128, 8], FP32, kind="ExternalInput")
    tri_d = nc.dram_tensor("tri", [128, 2, 128], BF16, kind="ExternalInput")
    bv_d = nc.dram_tensor("bv", [1, NG], FP32, kind="ExternalInput")
    wo_d = nc.dram_tensor("wo", [128, 4, D], R32, kind="ExternalInput")
    out_d = nc.dram_tensor("out", [S, D], FP32, kind="ExternalOutput")

    with TileContext(nc) as tc:
        with (
            tc.tile_pool(name="const", bufs=1) as const,
            tc.tile_pool(name="persist", bufs=1) as persist,
            tc.tile_pool(name="work", bufs=2) as work,
            tc.tile_pool(name="psum", bufs=2, space="PSUM") as psum,
        ):
            xT = {}      # strip -> list of per-ds tiles

            def emit_xT_dmas(i):
                # x arrives pre-transposed from the host; one tile per ds so
                # the first projection matmul only waits for its own slice
                s0 = i * STRIP
                xT[i] = []
                for ds in range(DS):
                    t = work.tile([128, STRIP], R32, name=f"xT{ds}",
                                  tag=f"xT{ds}", bufs=2)
                    nc.sync.dma_start(
                        t, xt_d[ds * 128:(ds + 1) * 128, s0:s0 + STRIP])
                    xT[i].append(t)

            emit_xT_dmas(0)
            tri2 = const.tile([128, 2, 128], BF16, name="tri2")
            nc.sync.dma_start(tri2, tri_d[:, :, :])
            bqk_sb = const.tile([128, 8], FP32, name="bqk_sb")
            nc.sync.dma_start(bqk_sb, bqk_d[:, :])
            bv_sb = const.tile([1, NG], FP32, name="bv_sb")
            nc.sync.dma_start(bv_sb, bv_d[:, :])
            # bias broadcast for the Vn evacuation add (one-time, on gpsimd)
            bvb = const.tile([128, NG], FP32, name="bvb")
            nc.gpsimd.partition_broadcast(bvb, bv_sb[0:1, :])
            # weights pre-rearranged on host to [128, ...] (contiguous per
            # partition -> full-BW DMA, cheap descriptors). wq/wk stream as
            # interleaved halves on the Activation HWDGE queue so the first
            # K-projection unblocks early; wv rides the sync queue after
            # the strip-0 x tiles; wo (needed last) trails on the act queue
            wq_sb = []
            wk_sb = []
            for hf in range(2):
                wq_sb.append(const.tile([128, DS, NG // 2], R32,
                                        name=f"wq_sb{hf}"))
                nc.scalar.dma_start(wq_sb[hf], wq_d[:, hf])
                wk_sb.append(const.tile([128, DS, NG // 2], R32,
                                        name=f"wk_sb{hf}"))
                nc.scalar.dma_start(wk_sb[hf], wk_d[:, hf])
            wv_sb = const.tile([128, DS, NG], R32, name="wv_sb")
            nc.scalar.dma_start(wv_sb, wv_d[:, :, :])
            wo_sb = const.tile([128, 4, D], R32, name="wo_sb")
            nc.scalar.dma_start(wo_sb, wo_d[:, :, :])

            # Persistent zero-padded K^T per head and V tiles (both bf16)
            KTz = persist.tile([128, HPG, S], BF16, name="KTz")
            # even heads occupy rows 0-63 (zero 64-127); odd heads vice versa
            for h in range(HPG):
                zrow = 64 if h % 2 == 0 else 0
                nc.gpsimd.memset(KTz[zrow:zrow + 64, h, :], 0.0)
            Vn = persist.tile([128, S // 128, HPG, HD + 1], BF16, name="Vn")
            nc.gpsimd.memset(Vn[:, :, :, HD], 1.0)

            QT = {}      # strip -> tile
            OT = {}      # strip -> tile
            ob = {}      # strip -> tile

            def qk_chunk(i, which, nb):
                # 8 matmuls (full D contraction) + bias-add evacuation
                s0 = i * STRIP
                if which == 0 and nb == 0:
                    QT[i] = work.tile([128, 4, STRIP], BF16, name="QT",
                                      tag="QT", bufs=2)
                hf, c0 = nb // 2, (nb % 2) * 128
                w_sb = (wq_sb if which == 0 else wk_sb)[hf]
                ps = psum.tile([128, STRIP], FP32, name="ps", tag="ps_mm",
                               bufs=2)
                for ds in range(DS):
                    nc.tensor.matmul(
                        ps, lhsT=w_sb[:, ds, c0:c0 + 128],
                        rhs=xT[i][ds],
                        start=(ds == 0), stop=(ds == DS - 1))
                bcol = bqk_sb[:, 4 * which + nb:4 * which + nb + 1]
                if which == 0:
                    nc.vector.tensor_scalar_add(QT[i][:, nb, :], ps, bcol)
                else:
                    nc.vector.tensor_scalar_add(
                        KTz[0:64, 2 * nb, s0:s0 + STRIP],
                        ps[0:64, :], bcol[0:64, :])
                    nc.vector.tensor_scalar_add(
                        KTz[64:128, 2 * nb + 1, s0:s0 + STRIP],
                        ps[64:128, :], bcol[64:128, :])

            def v_chunk(i, st):
                stg = i * 4 + st
                ps = psum.tile([128, STRIP], FP32, name="psv", tag="ps_mm",
                               bufs=2)
                for ds in range(DS):
                    nc.tensor.matmul(
                        ps,
                        lhsT=xT[i][ds][:, st * 128:(st + 1) * 128],
                        rhs=wv_sb[:, ds],
                        start=(ds == 0), stop=(ds == DS - 1))
                nc.vector.tensor_add(
                    Vn[:, stg, :, 0:HD],
                    ps.rearrange("p (h d) -> p h d", d=HD),
                    bvb.rearrange("p (h d) -> p h d", d=HD))

            ops_open = {}  # (i, st, ec) -> psum tile across half-chunks

            def outproj_chunk(i, st, ec, nh, evac="v"):
                # half-chunk: ns 0-1 (nh=0) opens the psum group, ns 2-3
                # (nh=1) closes it and evacuates on gpsimd
                if (st, ec, nh) == (0, 0, 0) and i not in ob:
                    ob[i] = work.tile([128, D], FP32, name="ob", tag="ob",
                                      bufs=2)
                s0 = i * STRIP
                if nh == 0:
                    ps = psum.tile([128, STRIP], FP32, name="pso",
                                   tag="ps_mm", bufs=2)
                    ops_open[(i, st, ec)] = ps
                else:
                    ps = ops_open.pop((i, st, ec))
                for ns in (2 * nh, 2 * nh + 1):
                    nc.tensor.matmul(
                        ps,
                        lhsT=OT[i][:, ns, st * 128:(st + 1) * 128],
                        rhs=wo_sb[:, ns, ec * 512:(ec + 1) * 512],
                        start=(ns == 0), stop=(ns == 3))
                if nh == 1:
                    if evac == "s":
                        nc.scalar.copy(ob[i][:, ec * 512:(ec + 1) * 512], ps)
                    else:
                        nc.vector.tensor_copy(
                            ob[i][:, ec * 512:(ec + 1) * 512], ps)
                    if ec == 1:
                        nc.scalar.dma_start(
                            out_d[s0 + st * 128:s0 + (st + 1) * 128, :],
                            ob[i])

            def attention(i, fillers):
                def fill():
                    try:
                        next(fillers)()
                    except StopIteration:
                        pass

                npair = 2 * i + 2  # 2i off-diagonal pairs + 2 diagonal pairs
                OT[i] = work.tile([128, 4, STRIP], R32, name="OT", tag="OT",
                                  bufs=3)
                for h in range(HPG):
                    prow = (h % 2) * 64
                    nsub = h // 2
                    psO = psum.tile([128, STRIP], FP32, name="psO", tag="psO",
                                    bufs=2)
                    for p in range(npair):
                        psA = psum.tile([128, 1024], FP32, name="psA",
                                        tag="psA2", bufs=2)
                        expP = work.tile([128, 1024], BF16, name="expP",
                                         tag="expP", bufs=4)
                        if p < 2 * i:        # off-diagonal pair, full width
                            for l in range(2):
                                b = 2 * p + l
                                nc.tensor.matmul(
                                    psA[:, 512 * l:512 * (l + 1)],
                                    lhsT=KTz[:, h, b * 128:(b + 1) * 128],
                                    rhs=QT[i][:, nsub, :],
                                    start=True, stop=True)
                            nc.scalar.activation(expP, psA, AF.Exp,
                                                 scale=0.125)
                            for l in range(2):
                                b = 2 * p + l
                                nc.tensor.matmul(
                                    psO[0:HD + 1, :],
                                    lhsT=Vn[:, b, h, :],
                                    rhs=expP[:, 512 * l:512 * (l + 1)],
                                    start=(p == 0 and l == 0), stop=False,
                                    skip_group_check=True)
                        else:                # diagonal pair, causally trimmed
                            pd = p - 2 * i
                            for l in range(2):
                                j = 2 * pd + l
                                b = 4 * i + j
                                c0 = 512 * l + 128 * j
                                nc.tensor.matmul(
                                    psA[:, c0:512 * (l + 1)],
                                    lhsT=KTz[:, h, b * 128:(b + 1) * 128],
                                    rhs=QT[i][:, nsub, 128 * j:STRIP],
                                    start=True, stop=True,
                                    skip_group_check=True)
                            # one exp spanning both trimmed slots; the stale
                            # psum columns in between are exp'd but never read
                            nc.scalar.activation(
                                expP[:, 256 * pd:1024],
                                psA[:, 256 * pd:1024],
                                AF.Exp, scale=0.125)
                            # strided 2-corner causal mask multiply
                            cbase = expP[:, 256 * pd:256 * pd + 768]
                            cap = bass.AP(
                                tensor=cbase.tensor, offset=cbase.offset,
                                ap=[list(cbase.ap[0])] + [[640, 2], [1, 128]])
                            nc.vector.tensor_mul(cap, cap, tri2)
                            for l in range(2):
                                j = 2 * pd + l
                                b = 4 * i + j
                                c0 = 512 * l + 128 * j
                                nc.tensor.matmul(
                                    psO[0:HD + 1, 128 * j:STRIP],
                                    lhsT=Vn[:, b, h, :],
                                    rhs=expP[:, c0:512 * (l + 1)],
                                    start=(i == 0 and p == 0 and l == 0),
                                    stop=(p == npair - 1 and l == 1),
                                    skip_group_check=True)
                            fill()   # diag pairs are scalar-paced: extra fill
                        fill()
                    # normalize: recip of denominator row, broadcast, mult
                    den = work.tile([1, STRIP], FP32, name="den", tag="den",
                                    bufs=1)
                    nc.vector.tensor_copy(den, psO[HD:HD + 1, :])
                    recip = work.tile([1, STRIP], FP32, name="recip",
                                      tag="recip", bufs=1)
                    nc.vector.reciprocal_approx_fast(recip, den)
                    pbt = work.tile([64, STRIP], FP32, name="pbt", tag="pbt",
                                    bufs=2)
                    nc.gpsimd.partition_broadcast(pbt, recip[0:1, :])
                    nc.vector.tensor_mul(OT[i][prow:prow + 64, nsub, :],
                                         psO[0:HD, :], pbt)
                    fill()

            # ---- prologue: strip 0 projections ----
            for which in range(2):
                for nb in range(4):
                    qk_chunk(0, which, nb)
            for st in range(4):
                v_chunk(0, st)

            # ---- main loop: attention(i) with interleaved fillers ----
            for i in range(NSTRIP):
                fillers = []
                if i + 1 < NSTRIP:
                    emit_xT_dmas(i + 1)
                    for which in range(2):
                        for nb in range(4):
                            fillers.append(
                                lambda which=which, nb=nb:
                                qk_chunk(i + 1, which, nb))
                    for st in range(4):
                        fillers.append(lambda st=st: v_chunk(i + 1, st))
                # out-proj chunks available this strip: first half of the
                # previous strip's, deferred half of the one before (keeps
                # attention(3), which has no proj fillers, supplied with PE
                # work). Halves of one psum group stay adjacent.
                opc = []
                if i >= 1:
                    sts = (0, 1) if i < NSTRIP - 1 else (0, 1, 2, 3)
                    opc += [(i - 1, st, ec) for st in sts for ec in range(2)]
                if i >= 2:
                    opc += [(i - 2, st, ec) for st in (2, 3) for ec in range(2)]
                if opc:
                    mixed = []
                    fi = iter(fillers)
                    for ii, st, ec in opc:
                        mixed.append(lambda ii=ii, st=st, ec=ec:
                                     outproj_chunk(ii, st, ec, 0))
                        mixed.append(lambda ii=ii, st=st, ec=ec:
                                     outproj_chunk(ii, st, ec, 1))
                        for _ in range(2):
                            try:
                                mixed.append(next(fi))
                            except StopIteration:
                                break
                    mixed.extend(fi)
                    fillers = mixed
                fit = iter(fillers)
                attention(i, fit)
                for f in fit:   # leftover fillers
                    f()

            # ---- final strip out-projection (evacs split across
            # Scalar+Vector: both are idle here and the ps_mm rotation is
            # latency-bound on the evacuation) ----
            for st in range(4):
                for ec in range(2):
                    outproj_chunk(NSTRIP - 1, st, ec, 0)
                    outproj_chunk(NSTRIP - 1, st, ec, 1,
                                  evac="s" if ec == 0 else "v")
    nc.compile()
    return nc


_CACHE = {}


def _tri_mask():
    # T[p, l, c] = 1.0 if c >= p else 0 (keep sq >= sk on diagonal corners)
    p = np.arange(128)[:, None, None]
    c = np.arange(128)[None, None, :]
    return np.broadcast_to(
        (c >= p), (128, 2, 128)).astype(np.float32).astype(ml_dtypes.bfloat16)


def kernel(x, W_qkv, b_qkv, W_o, b_o):
    x = np.ascontiguousarray(np.asarray(x, dtype=np.float32))
    W_qkv = np.asarray(W_qkv, dtype=np.float32)
    b_qkv = np.asarray(b_qkv, dtype=np.float32)
    W_o = np.asarray(W_o, dtype=np.float32)
    b_o = np.asarray(b_o, dtype=np.float32)

    if "nc" not in _CACHE:
        _CACHE["nc"] = build_bass()
    nc = _CACHE["nc"]

    in_maps = []
    for c in range(N_CORES):
        b, g = c // G, c % G
        n0 = g * NG
        bq = b_qkv[n0:n0 + NG]
        bk = b_qkv[D + n0:D + n0 + NG]
        bqk = np.concatenate(
            [bq.reshape(4, 128).T, bk.reshape(4, 128).T], axis=1)  # [128, 8]
        def _w(m):  # [D, NG] -> [128, DS, NG] contiguous
            return np.ascontiguousarray(
                m.reshape(DS, 128, -1).transpose(1, 0, 2))

        def _wh(m):  # [D, NG] -> [128, 2, DS, NG//2] (n-halves contiguous)
            r = m.reshape(DS, 128, 2, NG // 2)
            return np.ascontiguousarray(r.transpose(1, 2, 0, 3))
        in_maps.append({
            "xt": np.ascontiguousarray(x[b].T),
            "wq": _wh(W_qkv[:, n0:n0 + NG]),
            "wk": _wh(W_qkv[:, D + n0:D + n0 + NG]),
            "wv": _w(W_qkv[:, 2 * D + n0:2 * D + n0 + NG]),
            "bqk": np.ascontiguousarray(bqk),
            "bv": np.ascontiguousarray(
                b_qkv[2 * D + n0:2 * D + n0 + NG].reshape(1, NG)),
            "wo": np.ascontiguousarray(
                W_o[n0:n0 + NG, :].reshape(4, 128, D).transpose(1, 0, 2)),
            "tri": _tri_mask(),
        })

    _CACHE["in_maps"] = in_maps
    res = run_bass_kernel_spmd(nc, in_maps, list(range(N_CORES)))
    outs = res.results

    out = np.empty((B, S, D), dtype=np.float32)
    for b in range(B):
        out[b] = outs[G * b]["out"] + outs[G * b + 1]["out"]
    out += b_o[None, None, :]
    return out


# revision 25
# speedup vs baseline: 1.0036x; 1.0016x over previous
"""Causal multi-head attention on 8 Trainium2 NeuronCores.

Problem (hardcoded): x [4, 2048, 1024] fp32, W_qkv [1024, 3072], b_qkv [3072],
W_o [1024, 1024], b_o [1024]; 16 heads, head_dim 64.

Sharding: 8 cores = 4 batches x 2 head-groups (8 heads each). Each core
computes QKV projection for its (batch, head-group), causal attention for its
8 heads, and a partial out-projection [2048, 1024]. Host sums the two
head-group partials per batch and adds b_o.

Kernel strategy (per core, "transposed" domain):
  - x strip [512, 1024] -> PE-transpose -> xT [128, 8ds, 512]
  - QT/KTz = W^T x^T via matmul(lhsT=W_tile, rhs=xT); KTz zero-padded per
    head so the score matmul contracts K=128 (keeps PE at full clock).
  - V natural = matmul(lhsT=xT_tile, rhs=Wv), stored [128, blk, head, 65]
    with a ones column (denominator accumulates in psO row 64).
  - Scores per (head, strip): sk-blocks processed in PAIRS sharing one
    [128,1024] 2-bank psum tile; ONE Exp activation per off-diagonal pair.
    Diagonal blocks are causally trimmed (A/exp/AV restricted to sq>=128j)
    and masked via one strided 2-corner multiply with a [128,128] triangle.
  - Normalize: denominator copy + reciprocal_approx_fast + gpsimd
    partition_broadcast; OT = psO * recip (DVE).
  - out partial = matmul(lhsT=OT tile, rhs=W_o tiles) -> [s, e] -> DMA out.
  - Software pipelining: transposes/QKV-proj of strip i+1 and out-proj of
    strip i-1 are interleaved as PE fillers between attention pairs, so the
    PE fills gaps while Scalar (Exp) paces the attention inner loop.
Projection/out-proj matmuls run float32r; attention matmuls run bf16.
"""

import ml_dtypes
import numpy as np

import concourse.bass as bass
from concourse import bacc
import concourse.mybir as mybir
from concourse.bass_utils import run_bass_kernel_spmd
from concourse.tile import TileContext

B, S, D = 4, 2048, 1024
H, HD = 16, 64
G = 2                  # head groups (cores per batch)
HPG = H // G           # 8 heads per core
NG = HPG * HD          # 512 qkv feature columns per core
N_CORES = 8
STRIP = 512            # sq strip width
NSTRIP = S // STRIP    # 4
DS = D // 128          # 8 contraction subtiles for the projections
FP32 = mybir.dt.float32
R32 = mybir.dt.float32r
BF16 = mybir.dt.bfloat16
AF = mybir.ActivationFunctionType


def build_bass(dbg=False):
    nc = bacc.Bacc("TRN2")

    xt_d = nc.dram_tensor("xt", [D, S], R32, kind="ExternalInput")
    wq_d = nc.dram_tensor("wq", [128, 2, DS, NG // 2], R32, kind="ExternalInput")
    wk_d = nc.dram_tensor("wk", [128, 2, DS, NG // 2], R32, kind="ExternalInput")
    wv_d = nc.dram_tensor("wv", [128, DS, NG], R32, kind="ExternalInput")
    bqk_d = nc.dram_tensor("bqk", [128, 8], FP32, kind="ExternalInput")
    tri_d = nc.dram_tensor("tri", [128, 2, 128], BF16, kind="ExternalInput")
    bv_d = nc.dram_tensor("bv", [1, NG], FP32, kind="ExternalInput")
    wo_d = nc.dram_tensor("wo", [128, 4, D], R32, kind="ExternalInput")
    out_d = nc.dram_tensor("out", [S, D], FP32, kind="ExternalOutput")

    with TileContext(nc) as tc:
        with (
            tc.tile_pool(name="const", bufs=1) as const,
            tc.tile_pool(name="persist", bufs=1) as persist,
            tc.tile_pool(name="work", bufs=2) as work,
            tc.tile_pool(name="psum", bufs=2, space="PSUM") as psum,
        ):
            xT = {}      # strip -> list of per-ds tiles

            def emit_xT_dmas(i):
                # x arrives pre-transposed from the host; one tile per ds so
                # the first projection matmul only waits for its own slice
                s0 = i * STRIP
                xT[i] = []
                for ds in range(DS):
                    t = work.tile([128, STRIP], R32, name=f"xT{ds}",
                                  tag=f"xT{ds}", bufs=2)
                    nc.sync.dma_start(
                        t, xt_d[ds * 128:(ds + 1) * 128, s0:s0 + STRIP])
                    xT[i].append(t)

            emit_xT_dmas(0)
            tri2 = const.tile([128, 2, 128], BF16, name="tri2")
            nc.sync.dma_start(tri2, tri_d[:, :, :])
            bqk_sb = const.tile([128, 8], FP32, name="bqk_sb")
            nc.sync.dma_start(bqk_sb, bqk_d[:, :])
            bv_sb = const.tile([1, NG], FP32, name="bv_sb")
            nc.sync.dma_start(bv_sb, bv_d[:, :])
            # bias broadcast for the Vn evacuation add (one-time, on gpsimd)
            bvb = const.tile([128, NG], FP32, name="bvb")
            nc.gpsimd.partition_broadcast(bvb, bv_sb[0:1, :])
            # weights pre-rearranged on host to [128, ...] (contiguous per
            # partition -> full-BW DMA, cheap descriptors). wq/wk stream as
            # interleaved halves on the Activation HWDGE queue so the first
            # K-projection unblocks early; wv rides the sync queue after
            # the strip-0 x tiles; wo (needed last) trails on the act queue
            wq_sb = [const.tile([128, DS, NG // 2], R32, name=f"wq_sb{h}")
                     for h in range(2)]
            wk_sb = [const.tile([128, DS, NG // 2], R32, name=f"wk_sb{h}")
                     for h in range(2)]
            wv_sb = const.tile([128, DS, NG], R32, name="wv_sb")
            wo_sb = const.tile([128, 4, D], R32, name="wo_sb")
            nc.scalar.dma_start(wq_sb[0], wq_d[:, 0])
            nc.scalar.dma_start(wk_sb[0], wk_d[:, 0])
            nc.scalar.dma_start(wv_sb, wv_d[:, :, :])
            nc.scalar.dma_start(wq_sb[1], wq_d[:, 1])
            nc.scalar.dma_start(wk_sb[1], wk_d[:, 1])
            nc.scalar.dma_start(wo_sb, wo_d[:, :, :])

            # Persistent zero-padded K^T per head and V tiles (both bf16)
            KTz = persist.tile([128, HPG, S], BF16, name="KTz")
            # even heads occupy rows 0-63 (zero 64-127); odd heads vice versa
            for h in range(HPG):
                zrow = 64 if h % 2 == 0 else 0
                nc.gpsimd.memset(KTz[zrow:zrow + 64, h, :], 0.0)
            Vn = persist.tile([128, S // 128, HPG, HD + 1], BF16, name="Vn")
            nc.gpsimd.memset(Vn[:, :, :, HD], 1.0)

            QT = {}      # strip -> tile
            OT = {}      # strip -> tile
            ob = {}      # strip -> tile

            def qk_chunk(i, which, nb):
                # 8 matmuls (full D contraction) + bias-add evacuation
                s0 = i * STRIP
                if which == 0 and nb == 0:
                    QT[i] = work.tile([128, 4, STRIP], BF16, name="QT",
                                      tag="QT", bufs=2)
                hf, c0 = nb // 2, (nb % 2) * 128
                w_sb = (wq_sb if which == 0 else wk_sb)[hf]
                ps = psum.tile([128, STRIP], FP32, name="ps", tag="ps_mm",
                               bufs=2)
                for ds in range(DS):
                    nc.tensor.matmul(
                        ps, lhsT=w_sb[:, ds, c0:c0 + 128],
                        rhs=xT[i][ds],
                        start=(ds == 0), stop=(ds == DS - 1))
                bcol = bqk_sb[:, 4 * which + nb:4 * which + nb + 1]
                if which == 0:
                    nc.vector.tensor_scalar_add(QT[i][:, nb, :], ps, bcol)
                else:
                    nc.vector.tensor_scalar_add(
                        KTz[0:64, 2 * nb, s0:s0 + STRIP],
                        ps[0:64, :], bcol[0:64, :])
                    nc.vector.tensor_scalar_add(
                        KTz[64:128, 2 * nb + 1, s0:s0 + STRIP],
                        ps[64:128, :], bcol[64:128, :])

            def v_chunk(i, st):
                stg = i * 4 + st
                ps = psum.tile([128, STRIP], FP32, name="psv", tag="ps_mm",
                               bufs=2)
                for ds in range(DS):
                    nc.tensor.matmul(
                        ps,
                        lhsT=xT[i][ds][:, st * 128:(st + 1) * 128],
                        rhs=wv_sb[:, ds],
                        start=(ds == 0), stop=(ds == DS - 1))
                nc.vector.tensor_add(
                    Vn[:, stg, :, 0:HD],
                    ps.rearrange("p (h d) -> p h d", d=HD),
                    bvb.rearrange("p (h d) -> p h d", d=HD))

            ops_open = {}  # (i, st, ec) -> psum tile across half-chunks

            def outproj_chunk(i, st, ec, nh, evac="v", ptag="ps_mm"):
                # half-chunk: ns 0-1 (nh=0) opens the psum group, ns 2-3
                # (nh=1) closes it and evacuates on gpsimd
                if (st, ec, nh) == (0, 0, 0) and i not in ob:
                    ob[i] = work.tile([128, D], FP32, name="ob", tag="ob",
                                      bufs=2)
                s0 = i * STRIP
                if nh == 0:
                    ps = psum.tile([128, STRIP], FP32, name="pso",
                                   tag=ptag, bufs=2)
                    ops_open[(i, st, ec)] = ps
                else:
                    ps = ops_open.pop((i, st, ec))
                for ns in (2 * nh, 2 * nh + 1):
                    nc.tensor.matmul(
                        ps,
                        lhsT=OT[i][:, ns, st * 128:(st + 1) * 128],
                        rhs=wo_sb[:, ns, ec * 512:(ec + 1) * 512],
                        start=(ns == 0), stop=(ns == 3))
                if nh == 1:
                    if evac == "s":
                        nc.scalar.copy(ob[i][:, ec * 512:(ec + 1) * 512], ps)
                    else:
                        nc.vector.tensor_copy(
                            ob[i][:, ec * 512:(ec + 1) * 512], ps)
                    if ec == 1:
                        nc.scalar.dma_start(
                            out_d[s0 + st * 128:s0 + (st + 1) * 128, :],
                            ob[i])

            def attention(i, fillers):
                def fill():
                    try:
                        next(fillers)()
                    except StopIteration:
                        pass

                npair = 2 * i + 2  # 2i off-diagonal pairs + 2 diagonal pairs
                OT[i] = work.tile([128, 4, STRIP], R32, name="OT", tag="OT",
                                  bufs=3)
                for h in range(HPG):
                    prow = (h % 2) * 64
                    nsub = h // 2
                    psO = psum.tile([128, STRIP], FP32, name="psO", tag="psO",
                                    bufs=2)
                    for p in range(npair):
                        psA = psum.tile([128, 1024], FP32, name="psA",
                                        tag="psA2", bufs=2)
                        expP = work.tile([128, 1024], BF16, name="expP",
                                         tag="expP", bufs=4)
                        if p < 2 * i:        # off-diagonal pair, full width
                            for l in range(2):
                                b = 2 * p + l
                                nc.tensor.matmul(
                                    psA[:, 512 * l:512 * (l + 1)],
                                    lhsT=KTz[:, h, b * 128:(b + 1) * 128],
                                    rhs=QT[i][:, nsub, :],
                                    start=True, stop=True)
                            nc.scalar.activation(expP, psA, AF.Exp,
                                                 scale=0.125)
                            for l in range(2):
                                b = 2 * p + l
                                nc.tensor.matmul(
                                    psO[0:HD + 1, :],
                                    lhsT=Vn[:, b, h, :],
                                    rhs=expP[:, 512 * l:512 * (l + 1)],
                                    start=(p == 0 and l == 0), stop=False,
                                    skip_group_check=True)
                        else:                # diagonal pair, causally trimmed
                            pd = p - 2 * i
                            for l in range(2):
                                j = 2 * pd + l
                                b = 4 * i + j
                                c0 = 512 * l + 128 * j
                                nc.tensor.matmul(
                                    psA[:, c0:512 * (l + 1)],
                                    lhsT=KTz[:, h, b * 128:(b + 1) * 128],
                                    rhs=QT[i][:, nsub, 128 * j:STRIP],
                                    start=True, stop=True,
                                    skip_group_check=True)
                            # one exp spanning both trimmed slots; the stale
                            # psum columns in between are exp'd but never read
                            nc.scalar.activation(
                                expP[:, 256 * pd:1024],
                                psA[:, 256 * pd:1024],
                                AF.Exp, scale=0.125)
                            # strided 2-corner causal mask multiply
                            cbase = expP[:, 256 * pd:256 * pd + 768]
                            cap = bass.AP(
                                tensor=cbase.tensor, offset=cbase.offset,
                                ap=[list(cbase.ap[0])] + [[640, 2], [1, 128]])
                            nc.vector.tensor_mul(cap, cap, tri2)
                            for l in range(2):
                                j = 2 * pd + l
                                b = 4 * i + j
                                c0 = 512 * l + 128 * j
                                nc.tensor.matmul(
                                    psO[0:HD + 1, 128 * j:STRIP],
                                    lhsT=Vn[:, b, h, :],
                                    rhs=expP[:, c0:512 * (l + 1)],
                                    start=(i == 0 and p == 0 and l == 0),
                                    stop=(p == npair - 1 and l == 1),
                                    skip_group_check=True)
                            fill()   # diag pairs are scalar-paced: extra fill
                        fill()
                    # normalize: recip of denominator row, broadcast, mult
                    den = work.tile([1, STRIP], FP32, name="den", tag="den",
                                    bufs=1)
                    nc.vector.tensor_copy(den, psO[HD:HD + 1, :])
                    recip = work.tile([1, STRIP], FP32, name="recip",
                                      tag="recip", bufs=1)
                    nc.vector.reciprocal_approx_fast(recip, den)
                    pbt = work.tile([64, STRIP], FP32, name="pbt", tag="pbt",
                                    bufs=2)
                    nc.gpsimd.partition_broadcast(pbt, recip[0:1, :])
                    nc.vector.tensor_mul(OT[i][prow:prow + 64, nsub, :],
                                         psO[0:HD, :], pbt)
                    fill()

            # ---- prologue: strip 0 projections, ordered to match DMA
            # arrival (half-0 weights, then wv, then half-1) ----
            for which in range(2):
                for nb in range(2):
                    qk_chunk(0, which, nb)
            for st in range(4):
                v_chunk(0, st)
            for which in range(2):
                for nb in range(2, 4):
                    qk_chunk(0, which, nb)

            # ---- main loop: attention(i) with interleaved fillers ----
            for i in range(NSTRIP):
                fillers = []
                if i + 1 < NSTRIP:
                    emit_xT_dmas(i + 1)
                    for which in range(2):
                        for nb in range(4):
                            fillers.append(
                                lambda which=which, nb=nb:
                                qk_chunk(i + 1, which, nb))
                    for st in range(4):
                        fillers.append(lambda st=st: v_chunk(i + 1, st))
                # out-proj chunks available this strip: first half of the
                # previous strip's, deferred half of the one before (keeps
                # attention(3), which has no proj fillers, supplied with PE
                # work). Halves of one psum group stay adjacent.
                opc = []
                if i >= 1:
                    sts = (0, 1) if i < NSTRIP - 1 else (0, 1, 2, 3)
                    opc += [(i - 1, st, ec) for st in sts for ec in range(2)]
                if i >= 2:
                    opc += [(i - 2, st, ec) for st in (2, 3) for ec in range(2)]
                if opc:
                    mixed = []
                    fi = iter(fillers)
                    for ii, st, ec in opc:
                        mixed.append(lambda ii=ii, st=st, ec=ec:
                                     outproj_chunk(ii, st, ec, 0))
                        mixed.append(lambda ii=ii, st=st, ec=ec:
                                     outproj_chunk(ii, st, ec, 1))
                        for _ in range(2):
                            try:
                                mixed.append(next(fi))
                            except StopIteration:
                                break
                    mixed.extend(fi)
                    fillers = mixed
                fit = iter(fillers)
                attention(i, fit)
                for f in fit:   # leftover fillers
                    f()

            # ---- final strip out-projection: evacs split across
            # Scalar+Vector and psum groups alternate between the (now idle)
            # psA2 banks and ps_mm, so 4 groups pipeline instead of 2 ----
            for st in range(4):
                for ec in range(2):
                    ptag = "psA2" if ec == 0 else "ps_mm"
                    outproj_chunk(NSTRIP - 1, st, ec, 0, ptag=ptag)
                    outproj_chunk(NSTRIP - 1, st, ec, 1,
                                  evac="s" if ec == 0 else "v", ptag=ptag)
    nc.compile()
    return nc


_CACHE = {}


def _tri_mask():
    # T[p, l, c] = 1.0 if c >= p else 0 (keep sq >= sk on diagonal corners)
    p = np.arange(128)[:, None, None]
    c = np.arange(128)[None, None, :]
    return np.broadcast_to(
        (c >= p), (128, 2, 128)).astype(np.float32).astype(ml_dtypes.bfloat16)


def kernel(x, W_qkv, b_qkv, W_o, b_o):
    x = np.ascontiguousarray(np.asarray(x, dtype=np.float32))
    W_qkv = np.asarray(W_qkv, dtype=np.float32)
    b_qkv = np.asarray(b_qkv, dtype=np.float32)
    W_o = np.asarray(W_o, dtype=np.float32)
    b_o = np.asarray(b_o, dtype=np.float32)

    if "nc" not in _CACHE:
        _CACHE["nc"] = build_bass()
    nc = _CACHE["nc"]

    in_maps = []
    for c in range(N_CORES):
        b, g = c // G, c % G
        n0 = g * NG
        bq = b_qkv[n0:n0 + NG]
        bk = b_qkv[D + n0:D + n0 + NG]
        bqk = np.concatenate(
            [bq.reshape(4, 128).T, bk.reshape(4, 128).T], axis=1)  # [128, 8]
        def _w(m):  # [D, NG] -> [128, DS, NG] contiguous
            return np.ascontiguousarray(
                m.reshape(DS, 128, -1).transpose(1, 0, 2))

        def _wh(m):  # [D, NG] -> [128, 2, DS, NG//2] (n-halves contiguous)
            r = m.reshape(DS, 128, 2, NG // 2)
            return np.ascontiguousarray(r.transpose(1, 2, 0, 3))
        in_maps.append({
            "xt": np.ascontiguousarray(x[b].T),
            "wq": _wh(W_qkv[:, n0:n0 + NG]),
            "wk": _wh(W_qkv[:, D + n0:D + n0 + NG]),
            "wv": _w(W_qkv[:, 2 * D + n0:2 * D + n0 + NG]),
            "bqk": np.ascontiguousarray(bqk),
            "bv": np.ascontiguousarray(
                b_qkv[2 * D + n0:2 * D + n0 + NG].reshape(1, NG)),
            "wo": np.ascontiguousarray(
                W_o[n0:n0 + NG, :].reshape(4, 128, D).transpose(1, 0, 2)),
            "tri": _tri_mask(),
        })

    _CACHE["in_maps"] = in_maps
    res = run_bass_kernel_spmd(nc, in_maps, list(range(N_CORES)))
    outs = res.results

    out = np.empty((B, S, D), dtype=np.float32)
    for b in range(B):
        out[b] = outs[G * b]["out"] + outs[G * b + 1]["out"]
    out += b_o[None, None, :]
    return out


# revision 27
# speedup vs baseline: 1.0061x; 1.0025x over previous
"""Causal multi-head attention on 8 Trainium2 NeuronCores.

Problem (hardcoded): x [4, 2048, 1024] fp32, W_qkv [1024, 3072], b_qkv [3072],
W_o [1024, 1024], b_o [1024]; 16 heads, head_dim 64.

Sharding: 8 cores = 4 batches x 2 head-groups (8 heads each). Each core
computes QKV projection for its (batch, head-group), causal attention for its
8 heads, and a partial out-projection [2048, 1024]. Host sums the two
head-group partials per batch and adds b_o.

Kernel strategy (per core, "transposed" domain):
  - x strip [512, 1024] -> PE-transpose -> xT [128, 8ds, 512]
  - QT/KTz = W^T x^T via matmul(lhsT=W_tile, rhs=xT); KTz zero-padded per
    head so the score matmul contracts K=128 (keeps PE at full clock).
  - V natural = matmul(lhsT=xT_tile, rhs=Wv), stored [128, blk, head, 65]
    with a ones column (denominator accumulates in psO row 64).
  - Scores per (head, strip): sk-blocks processed in PAIRS sharing one
    [128,1024] 2-bank psum tile; ONE Exp activation per off-diagonal pair.
    Diagonal blocks are causally trimmed (A/exp/AV restricted to sq>=128j)
    and masked via one strided 2-corner multiply with a [128,128] triangle.
  - Normalize: denominator copy + reciprocal_approx_fast + gpsimd
    partition_broadcast; OT = psO * recip (DVE).
  - out partial = matmul(lhsT=OT tile, rhs=W_o tiles) -> [s, e] -> DMA out.
  - Software pipelining: transposes/QKV-proj of strip i+1 and out-proj of
    strip i-1 are interleaved as PE fillers between attention pairs, so the
    PE fills gaps while Scalar (Exp) paces the attention inner loop.
Projection/out-proj matmuls run float32r; attention matmuls run bf16.
"""

import ml_dtypes
import numpy as np

import concourse.bass as bass
from concourse import bacc
import concourse.mybir as mybir
from concourse.bass_utils import run_bass_kernel_spmd
from concourse.tile import TileContext

B, S, D = 4, 2048, 1024
H, HD = 16, 64
G = 2                  # head groups (cores per batch)
HPG = H // G           # 8 heads per core
NG = HPG * HD          # 512 qkv feature columns per core
N_CORES = 8
STRIP = 512            # sq strip width
NSTRIP = S // STRIP    # 4
DS = D // 128          # 8 contraction subtiles for the projections
FP32 = mybir.dt.float32
R32 = mybir.dt.float32r
BF16 = mybir.dt.bfloat16
AF = mybir.ActivationFunctionType


def build_bass(dbg=False):
    nc = bacc.Bacc("TRN2")

    xt_d = nc.dram_tensor("xt", [D, S], R32, kind="ExternalInput")
    wq_d = nc.dram_tensor("wq", [128, 2, DS, NG // 2], R32, kind="ExternalInput")
    wk_d = nc.dram_tensor("wk", [128, 2, DS, NG // 2], R32, kind="ExternalInput")
    wv_d = nc.dram_tensor("wv", [128, DS, NG], R32, kind="ExternalInput")
    bqk_d = nc.dram_tensor("bqk", [128, 8], FP32, kind="ExternalInput")
    tri_d = nc.dram_tensor("tri", [128, 2, 128], BF16, kind="ExternalInput")
    bv_d = nc.dram_tensor("bv", [1, NG], FP32, kind="ExternalInput")
    wo_d = nc.dram_tensor("wo", [128, 4, D], R32, kind="ExternalInput")
    out_d = nc.dram_tensor("out", [S, D], FP32, kind="ExternalOutput")

    with TileContext(nc) as tc:
        with (
            tc.tile_pool(name="const", bufs=1) as const,
            tc.tile_pool(name="persist", bufs=1) as persist,
            tc.tile_pool(name="work", bufs=2) as work,
            tc.tile_pool(name="psum", bufs=2, space="PSUM") as psum,
        ):
            xT = {}      # strip -> list of per-ds tiles

            def emit_xT_dmas(i):
                # x arrives pre-transposed from the host; one tile per ds so
                # the first projection matmul only waits for its own slice
                s0 = i * STRIP
                xT[i] = []
                for ds in range(DS):
                    t = work.tile([128, STRIP], R32, name=f"xT{ds}",
                                  tag=f"xT{ds}", bufs=2)
                    nc.sync.dma_start(
                        t, xt_d[ds * 128:(ds + 1) * 128, s0:s0 + STRIP])
                    xT[i].append(t)

            emit_xT_dmas(0)
            tri2 = const.tile([128, 2, 128], BF16, name="tri2")
            nc.sync.dma_start(tri2, tri_d[:, :, :])
            bqk_sb = const.tile([128, 8], FP32, name="bqk_sb")
            nc.sync.dma_start(bqk_sb, bqk_d[:, :])
            bv_sb = const.tile([1, NG], FP32, name="bv_sb")
            nc.sync.dma_start(bv_sb, bv_d[:, :])
            # bias broadcast for the Vn evacuation add (one-time, on gpsimd)
            bvb = const.tile([128, NG], FP32, name="bvb")
            nc.gpsimd.partition_broadcast(bvb, bv_sb[0:1, :])

            # weights pre-rearranged on host to [128, ...] (contiguous per
            # partition -> full-BW DMA, cheap descriptors). wq/wk stream as
            # interleaved halves on the Activation HWDGE queue so the first
            # K-projection unblocks early; wv rides the sync queue after
            # the strip-0 x tiles; wo (needed last) trails on the act queue
            wq_sb = [const.tile([128, DS, NG // 2], R32, name=f"wq_sb{h}")
                     for h in range(2)]
            wk_sb = [const.tile([128, DS, NG // 2], R32, name=f"wk_sb{h}")
                     for h in range(2)]
            wv_sb = const.tile([128, DS, NG], R32, name="wv_sb")
            wo_sb = const.tile([128, 4, D], R32, name="wo_sb")
            nc.scalar.dma_start(wq_sb[0], wq_d[:, 0])
            nc.scalar.dma_start(wk_sb[0], wk_d[:, 0])
            nc.scalar.dma_start(wv_sb, wv_d[:, :, :])
            nc.scalar.dma_start(wq_sb[1], wq_d[:, 1])
            nc.scalar.dma_start(wk_sb[1], wk_d[:, 1])
            nc.scalar.dma_start(wo_sb, wo_d[:, :, :])

            # Persistent zero-padded K^T per head and V tiles (both bf16)
            KTz = persist.tile([128, HPG, S], BF16, name="KTz")
            # even heads occupy rows 0-63 (zero 64-127); odd heads vice versa
            for h in range(HPG):
                zrow = 64 if h % 2 == 0 else 0
                nc.gpsimd.memset(KTz[zrow:zrow + 64, h, :], 0.0)
            Vn = persist.tile([128, S // 128, HPG, HD + 1], BF16, name="Vn")
            nc.gpsimd.memset(Vn[:, :, :, HD], 1.0)

            QT = {}      # strip -> tile
            OT = {}      # strip -> tile
            ob = {}      # strip -> tile

            def qk_chunk(i, which, nb):
                # 8 matmuls (full D contraction) + bias-add evacuation
                s0 = i * STRIP
                if which == 0 and nb == 0:
                    QT[i] = work.tile([128, 4, STRIP], BF16, name="QT",
                                      tag="QT", bufs=2)
                hf, c0 = nb // 2, (nb % 2) * 128
                w_sb = (wq_sb if which == 0 else wk_sb)[hf]
                ps = psum.tile([128, STRIP], FP32, name="ps", tag="ps_mm",
                               bufs=2)
                for ds in range(DS):
                    nc.tensor.matmul(
                        ps, lhsT=w_sb[:, ds, c0:c0 + 128],
                        rhs=xT[i][ds],
                        start=(ds == 0), stop=(ds == DS - 1))
                bcol = bqk_sb[:, 4 * which + nb:4 * which + nb + 1]
                if which == 0:
                    nc.vector.tensor_scalar_add(QT[i][:, nb, :], ps, bcol)
                else:
                    nc.vector.tensor_scalar_add(
                        KTz[0:64, 2 * nb, s0:s0 + STRIP],
                        ps[0:64, :], bcol[0:64, :])
                    nc.vector.tensor_scalar_add(
                        KTz[64:128, 2 * nb + 1, s0:s0 + STRIP],
                        ps[64:128, :], bcol[64:128, :])

            def v_chunk(i, st):
                stg = i * 4 + st
                ps = psum.tile([128, STRIP], FP32, name="psv", tag="ps_mm",
                               bufs=2)
                for ds in range(DS):
                    nc.tensor.matmul(
                        ps,
                        lhsT=xT[i][ds][:, st * 128:(st + 1) * 128],
                        rhs=wv_sb[:, ds],
                        start=(ds == 0), stop=(ds == DS - 1))
                nc.vector.tensor_add(
                    Vn[:, stg, :, 0:HD],
                    ps.rearrange("p (h d) -> p h d", d=HD),
                    bvb.rearrange("p (h d) -> p h d", d=HD))

            ops_open = {}  # (i, st, ec) -> psum tile across half-chunks

            def outproj_chunk(i, st, ec, nh, evac="v", ptag="ps_mm"):
                # half-chunk: ns 0-1 (nh=0) opens the psum group, ns 2-3
                # (nh=1) closes it and evacuates on gpsimd
                if (st, ec, nh) == (0, 0, 0) and i not in ob:
                    ob[i] = work.tile([128, D], FP32, name="ob", tag="ob",
                                      bufs=2)
                s0 = i * STRIP
                if nh == 0:
                    ps = psum.tile([128, STRIP], FP32, name="pso",
                                   tag=ptag, bufs=2)
                    ops_open[(i, st, ec)] = ps
                else:
                    ps = ops_open.pop((i, st, ec))
                for ns in (2 * nh, 2 * nh + 1):
                    nc.tensor.matmul(
                        ps,
                        lhsT=OT[i][:, ns, st * 128:(st + 1) * 128],
                        rhs=wo_sb[:, ns, ec * 512:(ec + 1) * 512],
                        start=(ns == 0), stop=(ns == 3))
                if nh == 1:
                    if evac == "s":
                        nc.scalar.copy(ob[i][:, ec * 512:(ec + 1) * 512], ps)
                    else:
                        nc.vector.tensor_copy(
                            ob[i][:, ec * 512:(ec + 1) * 512], ps)
                    if ec == 1:
                        nc.scalar.dma_start(
                            out_d[s0 + st * 128:s0 + (st + 1) * 128, :],
                            ob[i])

            def attention(i, fillers):
                def fill():
                    try:
                        next(fillers)()
                    except StopIteration:
                        pass

                npair = 2 * i + 2  # 2i off-diagonal pairs + 2 diagonal pairs
                OT[i] = work.tile([128, 4, STRIP], R32, name="OT", tag="OT",
                                  bufs=3)
                for h in range(HPG):
                    prow = (h % 2) * 64
                    nsub = h // 2
                    psO = psum.tile([128, STRIP], FP32, name="psO", tag="psO",
                                    bufs=2)
                    for p in range(npair):
                        psA = psum.tile([128, 1024], FP32, name="psA",
                                        tag="psA2", bufs=2)
                        expP = work.tile([128, 1024], BF16, name="expP",
                                         tag="expP", bufs=4)
                        if p < 2 * i:        # off-diagonal pair, full width
                            for l in range(2):
                                b = 2 * p + l
                                nc.tensor.matmul(
                                    psA[:, 512 * l:512 * (l + 1)],
                                    lhsT=KTz[:, h, b * 128:(b + 1) * 128],
                                    rhs=QT[i][:, nsub, :],
                                    start=True, stop=True)
                            nc.scalar.activation(expP, psA, AF.Exp,
                                                 scale=0.125)
                            for l in range(2):
                                b = 2 * p + l
                                nc.tensor.matmul(
                                    psO[0:HD + 1, :],
                                    lhsT=Vn[:, b, h, :],
                                    rhs=expP[:, 512 * l:512 * (l + 1)],
                                    start=(p == 0 and l == 0), stop=False,
                                    skip_group_check=True)
                        else:                # diagonal pair, causally trimmed
                            pd = p - 2 * i
                            for l in range(2):
                                j = 2 * pd + l
                                b = 4 * i + j
                                c0 = 512 * l + 128 * j
                                nc.tensor.matmul(
                                    psA[:, c0:512 * (l + 1)],
                                    lhsT=KTz[:, h, b * 128:(b + 1) * 128],
                                    rhs=QT[i][:, nsub, 128 * j:STRIP],
                                    start=True, stop=True,
                                    skip_group_check=True)
                            # one exp spanning both trimmed slots; the stale
                            # psum columns in between are exp'd but never read
                            nc.scalar.activation(
                                expP[:, 256 * pd:1024],
                                psA[:, 256 * pd:1024],
                                AF.Exp, scale=0.125)
                            # strided 2-corner causal mask multiply
                            cbase = expP[:, 256 * pd:256 * pd + 768]
                            cap = bass.AP(
                                tensor=cbase.tensor, offset=cbase.offset,
                                ap=[list(cbase.ap[0])] + [[640, 2], [1, 128]])
                            nc.vector.tensor_mul(cap, cap, tri2)
                            for l in range(2):
                                j = 2 * pd + l
                                b = 4 * i + j
                                c0 = 512 * l + 128 * j
                                nc.tensor.matmul(
                                    psO[0:HD + 1, 128 * j:STRIP],
                                    lhsT=Vn[:, b, h, :],
                                    rhs=expP[:, c0:512 * (l + 1)],
                                    start=(i == 0 and p == 0 and l == 0),
                                    stop=(p == npair - 1 and l == 1),
                                    skip_group_check=True)
                            fill()   # diag pairs are scalar-paced: extra fill
                        fill()
                    # normalize: recip of denominator row, broadcast, mult
                    den = work.tile([1, STRIP], FP32, name="den", tag="den",
                                    bufs=1)
                    nc.vector.tensor_copy(den, psO[HD:HD + 1, :])
                    recip = work.tile([1, STRIP], FP32, name="recip",
                                      tag="recip", bufs=1)
                    nc.vector.reciprocal_approx_fast(recip, den)
                    pbt = work.tile([64, STRIP], FP32, name="pbt", tag="pbt",
                                    bufs=2)
                    nc.gpsimd.partition_broadcast(pbt, recip[0:1, :])
                    nc.vector.tensor_mul(OT[i][prow:prow + 64, nsub, :],
                                         psO[0:HD, :], pbt)
                    fill()

            # ---- prologue: strip 0 projections, ordered to match DMA
            # arrival (half-0 weights, then wv, then half-1) ----
            for which in range(2):
                for nb in range(2):
                    qk_chunk(0, which, nb)
            for st in range(4):
                v_chunk(0, st)
            for which in range(2):
                for nb in range(2, 4):
                    qk_chunk(0, which, nb)

            # ---- main loop: attention(i) with interleaved fillers ----
            for i in range(NSTRIP):
                fillers = []
                if i + 1 < NSTRIP:
                    emit_xT_dmas(i + 1)
                    for which in range(2):
                        for nb in range(4):
                            fillers.append(
                                lambda which=which, nb=nb:
                                qk_chunk(i + 1, which, nb))
                    for st in range(4):
                        fillers.append(lambda st=st: v_chunk(i + 1, st))
                # out-proj chunks available this strip: first half of the
                # previous strip's, deferred half of the one before (keeps
                # attention(3), which has no proj fillers, supplied with PE
                # work). Halves of one psum group stay adjacent.
                opc = []
                if i >= 1:
                    sts = (0, 1) if i < NSTRIP - 1 else (0, 1, 2, 3)
                    opc += [(i - 1, st, ec) for st in sts for ec in range(2)]
                if i >= 2:
                    opc += [(i - 2, st, ec) for st in (2, 3) for ec in range(2)]
                if opc:
                    mixed = []
                    fi = iter(fillers)
                    for ii, st, ec in opc:
                        mixed.append(lambda ii=ii, st=st, ec=ec:
                                     outproj_chunk(ii, st, ec, 0))
                        mixed.append(lambda ii=ii, st=st, ec=ec:
                                     outproj_chunk(ii, st, ec, 1))
                        for _ in range(2):
                            try:
                                mixed.append(next(fi))
                            except StopIteration:
                                break
                    mixed.extend(fi)
                    fillers = mixed
                fit = iter(fillers)
                attention(i, fit)
                for f in fit:   # leftover fillers
                    f()

            # ---- final strip out-projection: evacs split across
            # Scalar+Vector and psum groups alternate between the (now idle)
            # psA2 banks and ps_mm, so 4 groups pipeline instead of 2 ----
            for st in range(4):
                for ec in range(2):
                    ptag = "psA2" if ec == 0 else "ps_mm"
                    outproj_chunk(NSTRIP - 1, st, ec, 0, ptag=ptag)
                    outproj_chunk(NSTRIP - 1, st, ec, 1,
                                  evac="s" if ec == 0 else "v", ptag=ptag)
    nc.compile()
    return nc


_CACHE = {}


def _tri_mask():
    # T[p, l, c] = 1.0 if c >= p else 0 (keep sq >= sk on diagonal corners)
    p = np.arange(128)[:, None, None]
    c = np.arange(128)[None, None, :]
    return np.broadcast_to(
        (c >= p), (128, 2, 128)).astype(np.float32).astype(ml_dtypes.bfloat16)


def kernel(x, W_qkv, b_qkv, W_o, b_o):
    x = np.ascontiguousarray(np.asarray(x, dtype=np.float32))
    W_qkv = np.asarray(W_qkv, dtype=np.float32)
    b_qkv = np.asarray(b_qkv, dtype=np.float32)
    W_o = np.asarray(W_o, dtype=np.float32)
    b_o = np.asarray(b_o, dtype=np.float32)

    if "nc" not in _CACHE:
        _CACHE["nc"] = build_bass()
    nc = _CACHE["nc"]

    in_maps = []
    for c in range(N_CORES):
        b, g = c // G, c % G
        n0 = g * NG
        bq = b_qkv[n0:n0 + NG]
        bk = b_qkv[D + n0:D + n0 + NG]
        bqk = np.concatenate(
            [bq.reshape(4, 128).T, bk.reshape(4, 128).T], axis=1)  # [128, 8]
        def _w(m):  # [D, NG] -> [128, DS, NG] contiguous
            return np.ascontiguousarray(
                m.reshape(DS, 128, -1).transpose(1, 0, 2))

        def _wh(m):  # [D, NG] -> [128, 2, DS, NG//2] (n-halves contiguous)
            r = m.reshape(DS, 128, 2, NG // 2)
            return np.ascontiguousarray(r.transpose(1, 2, 0, 3))
        in_maps.append({
            "xt": np.ascontiguousarray(x[b].T),
            "wq": _wh(W_qkv[:, n0:n0 + NG]),
            "wk": _wh(W_qkv[:, D + n0:D + n0 + NG]),
            "wv": _w(W_qkv[:, 2 * D + n0:2 * D + n0 + NG]),
            "bqk": np.ascontiguousarray(bqk),
            "bv": np.ascontiguousarray(
                b_qkv[2 * D + n0:2 * D + n0 + NG].reshape(1, NG)),
            "wo": np.ascontiguousarray(
                W_o[n0:n0 + NG, :].reshape(4, 128, D).transpose(1, 0, 2)),
            "tri": _tri_mask(),
        })

    _CACHE["in_maps"] = in_maps
    res = run_bass_kernel_spmd(nc, in_maps, list(range(N_CORES)))
    outs = res.results

    out = np.empty((B, S, D), dtype=np.float32)
    for b in range(B):
        out[b] = outs[G * b]["out"] + outs[G * b + 1]["out"]
    out += b_o[None, None, :]
    return out


# revision 28
# speedup vs baseline: 1.0179x; 1.0117x over previous
"""Causal multi-head attention on 8 Trainium2 NeuronCores.

Problem (hardcoded): x [4, 2048, 1024] fp32, W_qkv [1024, 3072], b_qkv [3072],
W_o [1024, 1024], b_o [1024]; 16 heads, head_dim 64.

Sharding: 8 cores = 4 batches x 2 head-groups (8 heads each). Each core
computes QKV projection for its (batch, head-group), causal attention for its
8 heads, and a partial out-projection [2048, 1024]. Host sums the two
head-group partials per batch and adds b_o.

Kernel strategy (per core, "transposed" domain):
  - x strip [512, 1024] -> PE-transpose -> xT [128, 8ds, 512]
  - QT/KTz = W^T x^T via matmul(lhsT=W_tile, rhs=xT); KTz zero-padded per
    head so the score matmul contracts K=128 (keeps PE at full clock).
  - V natural = matmul(lhsT=xT_tile, rhs=Wv), stored [128, blk, head, 65]
    with a ones column (denominator accumulates in psO row 64).
  - Scores per (head, strip): sk-blocks processed in PAIRS sharing one
    [128,1024] 2-bank psum tile; ONE Exp activation per off-diagonal pair.
    Diagonal blocks are causally trimmed (A/exp/AV restricted to sq>=128j)
    and masked via one strided 2-corner multiply with a [128,128] triangle.
  - Normalize: denominator copy + reciprocal_approx_fast + gpsimd
    partition_broadcast; OT = psO * recip (DVE).
  - out partial = matmul(lhsT=OT tile, rhs=W_o tiles) -> [s, e] -> DMA out.
  - Software pipelining: transposes/QKV-proj of strip i+1 and out-proj of
    strip i-1 are interleaved as PE fillers between attention pairs, so the
    PE fills gaps while Scalar (Exp) paces the attention inner loop.
Projection/out-proj matmuls run float32r; attention matmuls run bf16.
"""

import ml_dtypes
import numpy as np

import concourse.bass as bass
from concourse import bacc
import concourse.mybir as mybir
from concourse.bass_utils import run_bass_kernel_spmd
from concourse.tile import TileContext

B, S, D = 4, 2048, 1024
H, HD = 16, 64
G = 2                  # head groups (cores per batch)
HPG = H // G           # 8 heads per core
NG = HPG * HD          # 512 qkv feature columns per core
N_CORES = 8
STRIP = 512            # sq strip width
NSTRIP = S // STRIP    # 4
DS = D // 128          # 8 contraction subtiles for the projections
FP32 = mybir.dt.float32
R32 = mybir.dt.float32r
BF16 = mybir.dt.bfloat16
AF = mybir.ActivationFunctionType


def build_bass(dbg=False):
    nc = bacc.Bacc("TRN2")

    xt_d = nc.dram_tensor("xt", [D, S], R32, kind="ExternalInput")
    wq_d = nc.dram_tensor("wq", [128, 2, DS, NG // 2], R32, kind="ExternalInput")
    wk_d = nc.dram_tensor("wk", [128, 2, DS, NG // 2], R32, kind="ExternalInput")
    wv_d = nc.dram_tensor("wv", [128, DS, NG], R32, kind="ExternalInput")
    bqk_d = nc.dram_tensor("bqk", [128, 8], FP32, kind="ExternalInput")
    tri_d = nc.dram_tensor("tri", [128, 2, 128], BF16, kind="ExternalInput")
    bv_d = nc.dram_tensor("bv", [1, NG], FP32, kind="ExternalInput")
    wo_d = nc.dram_tensor("wo", [128, 4, D], R32, kind="ExternalInput")
    out_d = nc.dram_tensor("out", [S, D], FP32, kind="ExternalOutput")

    with TileContext(nc) as tc:
        with (
            tc.tile_pool(name="const", bufs=1) as const,
            tc.tile_pool(name="persist", bufs=1) as persist,
            tc.tile_pool(name="work", bufs=2) as work,
            tc.tile_pool(name="psum", bufs=2, space="PSUM") as psum,
        ):
            xT = {}      # strip -> list of per-ds tiles

            def emit_xT_dmas(i):
                # x arrives pre-transposed from the host; one tile per ds so
                # the first projection matmul only waits for its own slice
                s0 = i * STRIP
                xT[i] = []
                for ds in range(DS):
                    t = work.tile([128, STRIP], R32, name=f"xT{ds}",
                                  tag=f"xT{ds}", bufs=2)
                    nc.sync.dma_start(
                        t, xt_d[ds * 128:(ds + 1) * 128, s0:s0 + STRIP])
                    xT[i].append(t)

            emit_xT_dmas(0)
            tri2 = const.tile([128, 2, 128], BF16, name="tri2")
            nc.sync.dma_start(tri2, tri_d[:, :, :])
            bqk_sb = const.tile([128, 8], FP32, name="bqk_sb")
            nc.sync.dma_start(bqk_sb, bqk_d[:, :])
            bv_sb = const.tile([1, NG], FP32, name="bv_sb")
            nc.sync.dma_start(bv_sb, bv_d[:, :])
            # bias broadcast for the Vn evacuation add (one-time, on gpsimd)
            bvb = const.tile([128, NG], FP32, name="bvb")
            nc.gpsimd.partition_broadcast(bvb, bv_sb[0:1, :])

            # weights pre-rearranged on host to [128, ...] (contiguous per
            # partition -> full-BW DMA, cheap descriptors). wq/wk stream as
            # interleaved halves on the Activation HWDGE queue so the first
            # K-projection unblocks early; wv rides the sync queue after
            # the strip-0 x tiles; wo (needed last) trails on the act queue
            wq_sb = [const.tile([128, DS, NG // 2], R32, name=f"wq_sb{h}")
                     for h in range(2)]
            wk_sb = [const.tile([128, DS, NG // 2], R32, name=f"wk_sb{h}")
                     for h in range(2)]
            wv_sb = const.tile([128, DS, NG], R32, name="wv_sb")
            wo_sb = const.tile([128, 4, D], R32, name="wo_sb")
            nc.scalar.dma_start(wq_sb[0], wq_d[:, 0])
            nc.scalar.dma_start(wk_sb[0], wk_d[:, 0])
            nc.scalar.dma_start(wv_sb, wv_d[:, :, :])
            nc.scalar.dma_start(wq_sb[1], wq_d[:, 1])
            nc.scalar.dma_start(wk_sb[1], wk_d[:, 1])
            nc.scalar.dma_start(wo_sb, wo_d[:, :, :])

            # Persistent zero-padded K^T per head and V tiles (both bf16)
            KTz = persist.tile([128, HPG, S], BF16, name="KTz")
            # even heads occupy rows 0-63 (zero 64-127); odd heads vice versa
            for h in range(HPG):
                zrow = 64 if h % 2 == 0 else 0
                nc.gpsimd.memset(KTz[zrow:zrow + 64, h, :], 0.0)
            Vn = persist.tile([128, S // 128, HPG, HD + 1], BF16, name="Vn")
            nc.gpsimd.memset(Vn[:, :, :, HD], 1.0)

            QT = {}      # strip -> tile
            OT = {}      # strip -> tile
            ob = {}      # strip -> tile

            def qk_chunk(i, which, nb):
                # 8 matmuls (full D contraction) + bias-add evacuation
                s0 = i * STRIP
                if which == 0 and nb == 0:
                    QT[i] = work.tile([128, 4, STRIP], BF16, name="QT",
                                      tag="QT", bufs=2)
                hf, c0 = nb // 2, (nb % 2) * 128
                w_sb = (wq_sb if which == 0 else wk_sb)[hf]
                ps = psum.tile([128, STRIP], FP32, name="ps", tag="ps_mm",
                               bufs=2)
                for ds in range(DS):
                    nc.tensor.matmul(
                        ps, lhsT=w_sb[:, ds, c0:c0 + 128],
                        rhs=xT[i][ds],
                        start=(ds == 0), stop=(ds == DS - 1))
                bcol = bqk_sb[:, 4 * which + nb:4 * which + nb + 1]
                if which == 0:
                    nc.vector.tensor_scalar_add(QT[i][:, nb, :], ps, bcol)
                else:
                    nc.vector.tensor_scalar_add(
                        KTz[0:64, 2 * nb, s0:s0 + STRIP],
                        ps[0:64, :], bcol[0:64, :])
                    nc.vector.tensor_scalar_add(
                        KTz[64:128, 2 * nb + 1, s0:s0 + STRIP],
                        ps[64:128, :], bcol[64:128, :])

            def v_chunk(i, st):
                stg = i * 4 + st
                ps = psum.tile([128, STRIP], FP32, name="psv", tag="ps_mm",
                               bufs=2)
                for ds in range(DS):
                    nc.tensor.matmul(
                        ps,
                        lhsT=xT[i][ds][:, st * 128:(st + 1) * 128],
                        rhs=wv_sb[:, ds],
                        start=(ds == 0), stop=(ds == DS - 1))
                nc.vector.tensor_add(
                    Vn[:, stg, :, 0:HD],
                    ps.rearrange("p (h d) -> p h d", d=HD),
                    bvb.rearrange("p (h d) -> p h d", d=HD))

            ops_open = {}  # (i, st, ec) -> psum tile across half-chunks

            def outproj_chunk(i, st, ec, nh, evac="v", ptag="ps_mm"):
                # half-chunk: ns 0-1 (nh=0) opens the psum group, ns 2-3
                # (nh=1) closes it and evacuates on gpsimd
                if (st, ec, nh) == (0, 0, 0) and i not in ob:
                    ob[i] = work.tile([128, D], FP32, name="ob", tag="ob",
                                      bufs=2)
                s0 = i * STRIP
                if nh == 0:
                    ps = psum.tile([128, STRIP], FP32, name="pso",
                                   tag=ptag, bufs=2)
                    ops_open[(i, st, ec)] = ps
                else:
                    ps = ops_open.pop((i, st, ec))
                for ns in (2 * nh, 2 * nh + 1):
                    nc.tensor.matmul(
                        ps,
                        lhsT=OT[i][:, ns, st * 128:(st + 1) * 128],
                        rhs=wo_sb[:, ns, ec * 512:(ec + 1) * 512],
                        start=(ns == 0), stop=(ns == 3))
                if nh == 1:
                    if evac == "s":
                        nc.scalar.copy(ob[i][:, ec * 512:(ec + 1) * 512], ps)
                    else:
                        nc.vector.tensor_copy(
                            ob[i][:, ec * 512:(ec + 1) * 512], ps)
                    if ec == 1:
                        nc.scalar.dma_start(
                            out_d[s0 + st * 128:s0 + (st + 1) * 128, :],
                            ob[i])

            def attention(i, fillers):
                def fill():
                    try:
                        next(fillers)()
                    except StopIteration:
                        pass

                npair = 2 * i + 2  # 2i off-diagonal pairs + 2 diagonal pairs
                OT[i] = work.tile([128, 4, STRIP], R32, name="OT", tag="OT",
                                  bufs=3)
                for h in range(HPG):
                    prow = (h % 2) * 64
                    nsub = h // 2
                    psO = psum.tile([128, STRIP], FP32, name="psO", tag="psO",
                                    bufs=2)

                    def emit_avs(pend):
                        # AVs for an already-exp'd pair (one-pair lookahead:
                        # by now the exp/corner-mul are long done, so these
                        # issue without stalling the PE)
                        p, expP = pend
                        for l in range(2):
                            if p < 2 * i:
                                b, c0, j = 2 * p + l, 512 * l, 0
                            else:
                                j = 2 * (p - 2 * i) + l
                                b, c0 = 4 * i + j, 512 * l + 128 * j
                            nc.tensor.matmul(
                                psO[0:HD + 1, 128 * j:STRIP],
                                lhsT=Vn[:, b, h, :],
                                rhs=expP[:, c0:512 * (l + 1)],
                                start=(i == 0 and p == 0 and l == 0)
                                or (i > 0 and p == 0 and l == 0),
                                stop=(p == npair - 1 and l == 1),
                                skip_group_check=True)

                    pending = None
                    for p in range(npair):
                        psA = psum.tile([128, 1024], FP32, name="psA",
                                        tag="psA2", bufs=2)
                        expP = work.tile([128, 1024], BF16, name="expP",
                                         tag="expP", bufs=4)
                        if p < 2 * i:        # off-diagonal pair, full width
                            for l in range(2):
                                b = 2 * p + l
                                nc.tensor.matmul(
                                    psA[:, 512 * l:512 * (l + 1)],
                                    lhsT=KTz[:, h, b * 128:(b + 1) * 128],
                                    rhs=QT[i][:, nsub, :],
                                    start=True, stop=True)
                            nc.scalar.activation(expP, psA, AF.Exp,
                                                 scale=0.125)
                        else:                # diagonal pair, causally trimmed
                            pd = p - 2 * i
                            for l in range(2):
                                j = 2 * pd + l
                                b = 4 * i + j
                                c0 = 512 * l + 128 * j
                                nc.tensor.matmul(
                                    psA[:, c0:512 * (l + 1)],
                                    lhsT=KTz[:, h, b * 128:(b + 1) * 128],
                                    rhs=QT[i][:, nsub, 128 * j:STRIP],
                                    start=True, stop=True,
                                    skip_group_check=True)
                            # one exp spanning both trimmed slots; the stale
                            # psum columns in between are exp'd but never read
                            nc.scalar.activation(
                                expP[:, 256 * pd:1024],
                                psA[:, 256 * pd:1024],
                                AF.Exp, scale=0.125)
                            # strided 2-corner causal mask multiply
                            cbase = expP[:, 256 * pd:256 * pd + 768]
                            cap = bass.AP(
                                tensor=cbase.tensor, offset=cbase.offset,
                                ap=[list(cbase.ap[0])] + [[640, 2], [1, 128]])
                            nc.vector.tensor_mul(cap, cap, tri2)
                        if pending is not None:
                            emit_avs(pending)
                        pending = (p, expP)
                        fill()
                    emit_avs(pending)
                    # normalize: recip of denominator row, broadcast, mult
                    den = work.tile([1, STRIP], FP32, name="den", tag="den",
                                    bufs=1)
                    nc.vector.tensor_copy(den, psO[HD:HD + 1, :])
                    recip = work.tile([1, STRIP], FP32, name="recip",
                                      tag="recip", bufs=1)
                    nc.vector.reciprocal_approx_fast(recip, den)
                    pbt = work.tile([64, STRIP], FP32, name="pbt", tag="pbt",
                                    bufs=2)
                    nc.gpsimd.partition_broadcast(pbt, recip[0:1, :])
                    nc.vector.tensor_mul(OT[i][prow:prow + 64, nsub, :],
                                         psO[0:HD, :], pbt)
                    fill()

            # ---- prologue: strip 0 projections, ordered to match DMA
            # arrival (half-0 weights, then wv, then half-1) ----
            for which in range(2):
                for nb in range(2):
                    qk_chunk(0, which, nb)
            for st in range(4):
                v_chunk(0, st)
            for which in range(2):
                for nb in range(2, 4):
                    qk_chunk(0, which, nb)

            # ---- main loop: attention(i) with interleaved fillers ----
            for i in range(NSTRIP):
                fillers = []
                if i + 1 < NSTRIP:
                    emit_xT_dmas(i + 1)
                    for which in range(2):
                        for nb in range(4):
                            fillers.append(
                                lambda which=which, nb=nb:
                                qk_chunk(i + 1, which, nb))
                    for st in range(4):
                        fillers.append(lambda st=st: v_chunk(i + 1, st))
                # out-proj chunks available this strip: first half of the
                # previous strip's, deferred half of the one before (keeps
                # attention(3), which has no proj fillers, supplied with PE
                # work). Halves of one psum group stay adjacent.
                opc = []
                if i >= 1:
                    sts = (0, 1) if i < NSTRIP - 1 else (0, 1, 2, 3)
                    opc += [(i - 1, st, ec) for st in sts for ec in range(2)]
                if i >= 2:
                    opc += [(i - 2, st, ec) for st in (2, 3) for ec in range(2)]
                if opc:
                    mixed = []
                    fi = iter(fillers)
                    for ii, st, ec in opc:
                        mixed.append(lambda ii=ii, st=st, ec=ec:
                                     outproj_chunk(ii, st, ec, 0))
                        mixed.append(lambda ii=ii, st=st, ec=ec:
                                     outproj_chunk(ii, st, ec, 1))
                        for _ in range(2):
                            try:
                                mixed.append(next(fi))
                            except StopIteration:
                                break
                    mixed.extend(fi)
                    fillers = mixed
                fit = iter(fillers)
                attention(i, fit)
                for f in fit:   # leftover fillers
                    f()

            # ---- final strip out-projection: evacs split across
            # Scalar+Vector and psum groups alternate between the (now idle)
            # psA2 banks and ps_mm, so 4 groups pipeline instead of 2 ----
            for st in range(4):
                for ec in range(2):
                    ptag = "psA2" if ec == 0 else "ps_mm"
                    outproj_chunk(NSTRIP - 1, st, ec, 0, ptag=ptag)
                    outproj_chunk(NSTRIP - 1, st, ec, 1,
                                  evac="s" if ec == 0 else "v", ptag=ptag)
    nc.compile()
    return nc


_CACHE = {}


def _tri_mask():
    # T[p, l, c] = 1.0 if c >= p else 0 (keep sq >= sk on diagonal corners)
    p = np.arange(128)[:, None, None]
    c = np.arange(128)[None, None, :]
    return np.broadcast_to(
        (c >= p), (128, 2, 128)).astype(np.float32).astype(ml_dtypes.bfloat16)


def kernel(x, W_qkv, b_qkv, W_o, b_o):
    x = np.ascontiguousarray(np.asarray(x, dtype=np.float32))
    W_qkv = np.asarray(W_qkv, dtype=np.float32)
    b_qkv = np.asarray(b_qkv, dtype=np.float32)
    W_o = np.asarray(W_o, dtype=np.float32)
    b_o = np.asarray(b_o, dtype=np.float32)

    if "nc" not in _CACHE:
        _CACHE["nc"] = build_bass()
    nc = _CACHE["nc"]

    in_maps = []
    for c in range(N_CORES):
        b, g = c // G, c % G
        n0 = g * NG
        bq = b_qkv[n0:n0 + NG]
        bk = b_qkv[D + n0:D + n0 + NG]
        bqk = np.concatenate(
            [bq.reshape(4, 128).T, bk.reshape(4, 128).T], axis=1)  # [128, 8]
        def _w(m):  # [D, NG] -> [128, DS, NG] contiguous
            return np.ascontiguousarray(
                m.reshape(DS, 128, -1).transpose(1, 0, 2))

        def _wh(m):  # [D, NG] -> [128, 2, DS, NG//2] (n-halves contiguous)
            r = m.reshape(DS, 128, 2, NG // 2)
            return np.ascontiguousarray(r.transpose(1, 2, 0, 3))
        in_maps.append({
            "xt": np.ascontiguousarray(x[b].T),
            "wq": _wh(W_qkv[:, n0:n0 + NG]),
            "wk": _wh(W_qkv[:, D + n0:D + n0 + NG]),
            "wv": _w(W_qkv[:, 2 * D + n0:2 * D + n0 + NG]),
            "bqk": np.ascontiguousarray(bqk),
            "bv": np.ascontiguousarray(
                b_qkv[2 * D + n0:2 * D + n0 + NG].reshape(1, NG)),
            "wo": np.ascontiguousarray(
                W_o[n0:n0 + NG, :].reshape(4, 128, D).transpose(1, 0, 2)),
            "tri": _tri_mask(),
        })

    _CACHE["in_maps"] = in_maps
    res = run_bass_kernel_spmd(nc, in_maps, list(range(N_CORES)))
    outs = res.results

    out = np.empty((B, S, D), dtype=np.float32)
    for b in range(B):
        out[b] = outs[G * b]["out"] + outs[G * b + 1]["out"]
    out += b_o[None, None, :]
    return out
